# revision 1
# baseline (speedup 1.0000x reference)
"""Distributed Trainium2 (Bass/Tile) kernel for a Qwen3-style attention layer.

Full layer: QKV proj -> per-head RMSNorm (q,k) -> RoPE -> GQA SDPA -> o_proj.

Sharding over 8 NeuronCores:
  - tensor-parallel across heads for QKV+attention: core c owns q-heads
    [4c, 4c+4) and kv-head c; hidden_states replicated.
  - AllToAll exchanges attention context so each core ends with all 4096
    context dims for a 256-token slice; o_proj is then token-parallel with
    Wo replicated (streamed). Output: per-core [256, 4096] chunks that the
    host concatenates. No all-reduce needed.

Compute layout: everything lives transposed ([dim, token]) so the PE array
contracts over the partition axis with N=512 moving tiles in bf16.

Schedule: proj(b0) -> attn(b0) || A2A(b0) || proj(b1) -> [hid/weight pools
close] -> attn(b1) with its A2A split in two half-head collectives fired
mid-phase, Wo prefetch running underneath -> o_proj in 2-hid-group blocks,
batch-0 first (covers the tail of the last collective).
"""

import numpy as np
import ml_dtypes

import concourse.bass as bass
import concourse.mybir as mybir
from concourse import bacc
from concourse.tile import TileContext
from concourse.bass_utils import run_bass_kernel_spmd
from concourse.masks import make_identity

F32 = mybir.dt.float32
BF16 = mybir.dt.bfloat16
BF16_NP = ml_dtypes.bfloat16

N_CORES = 8

FULL_CFG = dict(B=2, S=1024, HID=4096, H=32, KV=8, D=128, eps=1e-6)


def build_program(B=2, S=1024, HID=4096, H=32, KV=8, D=128, eps=1e-6):
    cores = N_CORES
    assert D == 128 and H % cores == 0 and KV == cores and B == 2
    HQ = H // cores            # q heads per core
    HH = HQ // 2               # heads per a2a half (last batch)
    T = B * S                  # total tokens
    HCH = HID // 128           # hidden-dim chunks of 128
    TT = min(512, S)           # projection token tile (within batch)
    TPB = S // TT              # projection tiles per batch
    KB = S // 128              # key blocks per batch
    QT = min(512, S)           # attention q tile
    QTB = S // QT              # q tiles per batch
    TC = T // cores            # output tokens per core
    TCB = TC // B              # per-batch token slice per core
    ICH = (H * D) // 128       # o_proj contraction chunks (32)
    OH = min(512, HID // 2)    # o_proj hid tile width
    NHG = HID // OH            # number of hid groups
    GSZ = 2                    # hid groups per o_proj block
    assert NHG % GSZ == 0
    scale = float(D) ** -0.5
    MULT = mybir.AluOpType.mult
    SW = QTB * QT              # full q row per batch (== S)

    nc = bacc.Bacc("TRN2", target_bir_lowering=False, debug=False,
                   num_devices=cores)

    hT = nc.dram_tensor("hT", [B, HCH, 128, S], BF16, kind="ExternalInput")
    wq = nc.dram_tensor("wq", [HQ, 128, HCH * 128], BF16, kind="ExternalInput")
    wk = nc.dram_tensor("wk", [128, HCH * 128], BF16, kind="ExternalInput")
    wv = nc.dram_tensor("wv", [128, HCH * 128], BF16, kind="ExternalInput")
    wo = nc.dram_tensor("wo", [ICH, 128, HID], BF16, kind="ExternalInput")
    cosT = nc.dram_tensor("cosT", [128, S], BF16, kind="ExternalInput")
    csinT = nc.dram_tensor("csinT", [128, S], BF16, kind="ExternalInput")
    qw = nc.dram_tensor("qw", [128, 1], F32, kind="ExternalInput")
    kw = nc.dram_tensor("kw", [128, 1], F32, kind="ExternalInput")
    out = nc.dram_tensor("out", [TC, HID], F32, kind="ExternalOutput")

    with TileContext(nc) as tc:
        with (
            tc.tile_pool(name="const", bufs=1) as cp,
            tc.tile_pool(name="dram", bufs=1, space="DRAM") as dramp,
            tc.tile_pool(name="qkv", bufs=1) as p_qkv,
            tc.tile_pool(name="work", bufs=2) as p_work,
            tc.tile_pool(name="pt", bufs=2) as p_pt,
            tc.tile_pool(name="psum", bufs=1, space="PSUM") as ps_all,
        ):
            ones_s = cp.tile([128, 128], BF16)
            nc.vector.memset(ones_s[:, :], 1.0)
            ident = cp.tile([128, 128], BF16)
            make_identity(nc, ident[:, :])
            eps_s = cp.tile([128, 1], F32)
            nc.vector.memset(eps_s[:, :], eps)
            cos_s = cp.tile([128, S], BF16)
            nc.sync.dma_start(out=cos_s[:, :], in_=cosT[:, :])
            csin_s = cp.tile([128, S], BF16)
            nc.sync.dma_start(out=csin_s[:, :], in_=csinT[:, :])
            qw_s = cp.tile([128, 1], F32)
            nc.sync.dma_start(out=qw_s[:, :], in_=qw[:, :])
            kw_s = cp.tile([128, 1], F32)
            nc.sync.dma_start(out=kw_s[:, :], in_=kw[:, :])

            a2a0_in = dramp.tile([H * D, TCB], BF16, name="a2a0i")
            a2a0_out = dramp.tile([H * D, TCB], BF16, name="a2a0o")
            a2a1_in = [dramp.tile([cores * HH * 128, TCB], BF16,
                                  tag=f"a2a1i{p}", name=f"a2a1i{p}")
                       for p in range(2)]
            a2a1_out = [dramp.tile([cores * HH * 128, TCB], BF16,
                                   tag=f"a2a1o{p}", name=f"a2a1o{p}")
                        for p in range(2)]

            qT_s = p_qkv.tile([128, HQ * T], BF16, tag="qT")
            kT_s = p_qkv.tile([128, T], BF16, tag="kT")
            vnat_s = p_qkv.tile([128, T], BF16, tag="vnat")
            ctxT_s = p_qkv.tile([128, HQ * T], BF16, tag="ctxT")

            def proj(b, p_hid, p_w):
                """QKV projection + norm + rope for batch b."""
                w0_t = p_w.tile([128, HCH * 128], BF16, tag="w", name="w0")
                nc.sync.dma_start(out=w0_t[:, :], in_=wq[0])
                hch = []
                for ch in range(HCH):
                    t_ = p_hid.tile([128, S], BF16, tag="hid", name="hid")
                    nc.sync.dma_start(out=t_[:, :], in_=hT[b, ch, :, :])
                    hch.append(t_)
                for ob in range(HQ + 2):
                    if ob == 0:
                        w_t = w0_t
                    else:
                        w_t = p_w.tile([128, HCH * 128], BF16, tag="w",
                                       name="w")
                        srcw = (wq[ob] if ob < HQ else
                                (wk[:, :] if ob == HQ else wv[:, :]))
                        nc.sync.dma_start(out=w_t[:, :], in_=srcw)
                    for tt in range(TPB):
                        ps = ps_all.tile([128, TT], F32, tag="mm", name="ps", bufs=2, padded_shape=[128, SW])
                        for ch in range(HCH):
                            nc.tensor.matmul(
                                ps[:, :],
                                lhsT=w_t[:, ch * 128:(ch + 1) * 128],
                                rhs=hch[ch][:, tt * TT:(tt + 1) * TT],
                                start=(ch == 0), stop=(ch == HCH - 1))
                        tg = b * S + tt * TT
                        pos = tt * TT
                        if ob <= HQ:
                            is_q = ob < HQ
                            dst = (qT_s[:, ob * T + tg: ob * T + tg + TT]
                                   if is_q else kT_s[:, tg: tg + TT])
                            wcol = qw_s if is_q else kw_s
                            sq = p_work.tile([128, TT], BF16, tag="sq")
                            nc.scalar.square(sq[:, :], ps[:, :])
                            ssq = ps_all.tile([128, TT], F32, tag="aux", name="ssq", bufs=2)
                            nc.tensor.matmul(ssq[:, :], lhsT=ones_s[:, :],
                                             rhs=sq[:, :], start=True,
                                             stop=True)
                            std = p_work.tile([128, TT], F32, tag="std")
                            nc.scalar.activation(
                                std[:, :], ssq[:, :],
                                mybir.ActivationFunctionType.Sqrt,
                                bias=eps_s[:, :], scale=1.0 / D)
                            rs = p_work.tile([128, TT], F32, tag="rs")
                            nc.vector.reciprocal_approx_fast(rs[:, :],
                                                             std[:, :])
                            qn = p_work.tile([128, TT], F32, tag="qn")
                            nc.vector.scalar_tensor_tensor(
                                qn[:, :], in0=ps[:, :], scalar=wcol[:, :],
                                in1=rs[:, :], op0=MULT, op1=MULT)
                            qsw = p_work.tile([128, TT], F32, tag="qsw")
                            nc.sync.dma_start(out=qsw[0:64, :],
                                              in_=qn[64:128, :])
                            nc.sync.dma_start(out=qsw[64:128, :],
                                              in_=qn[0:64, :])
                            t1 = p_work.tile([128, TT], F32, tag="t1")
                            nc.vector.tensor_mul(t1[:, :], qn[:, :],
                                                 cos_s[:, pos: pos + TT])
                            t2 = p_work.tile([128, TT], BF16, tag="t2")
                            nc.vector.tensor_mul(t2[:, :], qsw[:, :],
                                                 csin_s[:, pos: pos + TT])
                            nc.vector.tensor_add(dst, t1[:, :], t2[:, :])
                        else:
                            vt = p_work.tile([128, TT], BF16, tag="vt")
                            nc.scalar.copy(vt[:, :], ps[:, :])
                            for tb in range(TT // 128):
                                vtr = ps_all.tile([128, 128], BF16, tag="aux", name="vtr", bufs=2)
                                nc.tensor.transpose(
                                    vtr[:, :], vt[:, tb * 128:(tb + 1) * 128],
                                    ident[:, :])
                                tbg = tg // 128 + tb
                                nc.scalar.copy(
                                    vnat_s[:, tbg * 128:(tbg + 1) * 128],
                                    vtr[:, :])

            def attn(b, mid_hook=None):
                """Attention for batch b + context shipping (A2A)."""
                last = b == B - 1
                for h in range(HQ):
                    qoff = h * T + b * S
                    pt_t = p_pt.tile([128, KB * SW], BF16, tag="pT",
                                     name="pT")
                    for kb in range(KB):
                        sps = ps_all.tile([128, SW], F32, tag="mm", name="sps", bufs=2)
                        for qt in range(QTB):
                            nc.tensor.matmul(
                                sps[:, qt * QT:(qt + 1) * QT],
                                lhsT=kT_s[:, b * S + kb * 128:
                                          b * S + (kb + 1) * 128],
                                rhs=qT_s[:, qoff + qt * QT:
                                         qoff + (qt + 1) * QT],
                                start=True, stop=True)
                        nc.scalar.activation(
                            pt_t[:, kb * SW:(kb + 1) * SW], sps[:, :],
                            mybir.ActivationFunctionType.Exp, scale=scale)
                    ctxs = [ps_all.tile([128, QT], F32, tag="ctx", name="ctx",
                                     bufs=2) for _ in range(QTB)]
                    for kb in range(KB):
                        tbg = (b * S) // 128 + kb
                        for qt in range(QTB):
                            nc.tensor.matmul(
                                ctxs[qt][:, :],
                                lhsT=vnat_s[:, tbg * 128:(tbg + 1) * 128],
                                rhs=pt_t[:, kb * SW + qt * QT:
                                         kb * SW + (qt + 1) * QT],
                                start=(kb == 0), stop=(kb == KB - 1))
                    denp = p_work.tile([128, SW], BF16, tag="denp")
                    dent = p_work.tile([128, SW], BF16, tag="dent")
                    if KB == 2:
                        nc.vector.tensor_add(denp[:, :], pt_t[:, 0:SW],
                                             pt_t[:, SW:2 * SW])
                    else:
                        assert KB % 4 == 0
                        nc.vector.tensor_add(denp[:, :], pt_t[:, 0:SW],
                                             pt_t[:, SW:2 * SW])
                        nc.vector.tensor_add(dent[:, :],
                                             pt_t[:, 2 * SW:3 * SW],
                                             pt_t[:, 3 * SW:4 * SW])
                        nc.vector.tensor_add(denp[:, :], denp[:, :],
                                             dent[:, :])
                        for g in range(1, KB // 4):
                            nc.vector.tensor_add(
                                dent[:, :],
                                pt_t[:, 4 * g * SW:(4 * g + 1) * SW],
                                pt_t[:, (4 * g + 1) * SW:(4 * g + 2) * SW])
                            nc.vector.tensor_add(denp[:, :], denp[:, :],
                                                 dent[:, :])
                            nc.vector.tensor_add(
                                dent[:, :],
                                pt_t[:, (4 * g + 2) * SW:(4 * g + 3) * SW],
                                pt_t[:, (4 * g + 3) * SW:(4 * g + 4) * SW])
                            nc.vector.tensor_add(denp[:, :], denp[:, :],
                                                 dent[:, :])
                    for qt in range(QTB):
                        dps = ps_all.tile([128, QT], F32, tag="aux", name="dps", bufs=2)
                        nc.tensor.matmul(dps[:, :], lhsT=ones_s[:, :],
                                         rhs=denp[:, qt * QT:(qt + 1) * QT],
                                         start=True, stop=True)
                        rec = p_work.tile([128, QT], F32, tag="rec")
                        nc.vector.reciprocal_approx_fast(rec[:, :], dps[:, :])
                        nc.vector.tensor_mul(
                            ctxT_s[:, qoff + qt * QT: qoff + (qt + 1) * QT],
                            ctxs[qt][:, :], rec[:, :])
                    # ship this head's context
                    if not last:
                        for j in range(cores):
                            nc.sync.dma_start(
                                out=a2a0_in[(j * HQ + h) * 128:
                                            (j * HQ + h + 1) * 128, :],
                                in_=ctxT_s[:, qoff + j * TCB:
                                           qoff + (j + 1) * TCB])
                    else:
                        pi, hh = h // HH, h % HH
                        for j in range(cores):
                            nc.sync.dma_start(
                                out=a2a1_in[pi][(j * HH + hh) * 128:
                                                (j * HH + hh + 1) * 128, :],
                                in_=ctxT_s[:, qoff + j * TCB:
                                           qoff + (j + 1) * TCB])
                        if hh == HH - 1:
                            nc.gpsimd.collective_compute(
                                "AllToAll", mybir.AluOpType.bypass,
                                replica_groups=[list(range(cores))],
                                ins=[a2a1_in[pi].opt()],
                                outs=[a2a1_out[pi].opt()])
                            if pi == 0 and mid_hook is not None:
                                mid_hook()
                if not last:
                    nc.gpsimd.collective_compute(
                        "AllToAll", mybir.AluOpType.bypass,
                        replica_groups=[list(range(cores))],
                        ins=[a2a0_in.opt()],
                        outs=[a2a0_out.opt()])

            # ---- phase 1: proj0, attn0 (+a2a0), proj1 (hid/w pools open) --
            with (
                tc.tile_pool(name="hid", bufs=HCH) as p_hid,
                tc.tile_pool(name="wts", bufs=2) as p_w,
            ):
                sc_ = nc.enter_named_scope("proj0", True)[0]
                proj(0, p_hid, p_w)
                nc.leave_named_scope("proj0", sc_, True)
                sc_ = nc.enter_named_scope("attn0", True)[0]
                attn(0)
                nc.leave_named_scope("attn0", sc_, True)
                sc_ = nc.enter_named_scope("proj1", True)[0]
                proj(1, p_hid, p_w)
                nc.leave_named_scope("proj1", sc_, True)

            # ---- phase 2: attn1 (split a2a) with Wo prefetch underneath --
            with (
                tc.tile_pool(name="wo", bufs=1) as p_wo,
                tc.tile_pool(name="cx", bufs=1) as p_cx,
                tc.tile_pool(name="oo", bufs=4) as p_oo,
            ):
                WOB = ICH + 2

                def load_wo_grp(hgs):
                    wts = {}
                    for ic in range(ICH):
                        wo_t = p_wo.tile([128, GSZ * OH], BF16, tag="wo",
                                         name="wo", bufs=WOB)
                        nc.sync.dma_start(
                            out=wo_t[:, :],
                            in_=wo[ic, :, hgs[0] * OH:
                                   (hgs[0] + GSZ) * OH])
                        for i, hg in enumerate(hgs):
                            wts[(hg, ic)] = wo_t[:, i * OH:(i + 1) * OH]
                    return wts

                # cx0 load can go early (a2a0 long done)
                cx_s = [p_cx.tile([128, ICH * TCB], BF16, tag=f"cx{b}",
                                  name=f"cx{b}") for b in range(B)]
                nc.sync.dma_start(
                    out=cx_s[0][:, :].rearrange("p (ic t) -> p ic t", ic=ICH),
                    in_=a2a0_out[:, :].rearrange("(ic p) t -> p ic t",
                                                 ic=ICH))
                grp0_hgs = [i for i in range(GSZ)]
                grp0_wts = {}

                def _mid_hook():
                    grp0_wts.update(load_wo_grp(grp0_hgs))

                sc_ = nc.enter_named_scope("attn1", True)[0]
                attn(1, mid_hook=_mid_hook)
                nc.leave_named_scope("attn1", sc_, True)

                sc_ = nc.enter_named_scope("oproj", True)[0]
                for grp in range(NHG // GSZ):
                    hgs = [grp * GSZ + i for i in range(GSZ)]
                    wts = grp0_wts if grp == 0 else load_wo_grp(hgs)
                    if grp == 0:
                        # cx1 from the two half-head pieces (after the
                        # first group's weight loads so they aren't
                        # blocked in the DMA queue behind the collective)
                        cxv = cx_s[1][:, :].rearrange(
                            "p (j four t) -> p j four t", four=HQ, t=TCB)
                        for pi in range(2):
                            srcv = a2a1_out[pi][:, :].rearrange(
                                "(j hh p) t -> p hh j t", hh=HH, p=128)
                            for hh in range(HH):
                                nc.sync.dma_start(
                                    out=cxv[:, :, pi * HH + hh, :],
                                    in_=srcv[:, hh])
                    for b in range(B):
                        for hg in hgs:
                            pso = ps_all.tile([TCB, OH], F32, tag="aux",
                                              name="pso", bufs=2)
                            for ic in range(ICH):
                                nc.tensor.matmul(
                                    pso[:, :],
                                    lhsT=cx_s[b][:, ic * TCB:(ic + 1) * TCB],
                                    rhs=wts[(hg, ic)][:, :],
                                    start=(ic == 0), stop=(ic == ICH - 1))
                            ot = p_oo.tile([TCB, OH], F32, tag="oout",
                                           name="oout")
                            nc.vector.tensor_copy(ot[:, :], pso[:, :])
                            nc.sync.dma_start(
                                out=out[b * TCB:(b + 1) * TCB,
                                        hg * OH:(hg + 1) * OH],
                                in_=ot[:, :])
                nc.leave_named_scope("oproj", sc_, True)

    nc.compile()
    return nc


def host_prep(inputs, B=2, S=1024, HID=4096, H=32, KV=8, D=128, eps=1e-6):
    """Shard + lay out the full inputs into per-core in_maps."""
    cores = N_CORES
    HQ = H // cores
    T = B * S
    HCH = HID // 128
    ICH = (H * D) // 128

    hs = np.ascontiguousarray(inputs["hidden_states"], dtype=np.float32)
    fc = np.asarray(inputs["freqs_cis"], dtype=np.float32)
    Wq = np.asarray(inputs["Wq"], dtype=np.float32)
    Wk = np.asarray(inputs["Wk"], dtype=np.float32)
    Wv = np.asarray(inputs["Wv"], dtype=np.float32)
    Wo = np.asarray(inputs["Wo"], dtype=np.float32)
    qnw = np.asarray(inputs["q_norm_w"], dtype=np.float32)
    knw = np.asarray(inputs["k_norm_w"], dtype=np.float32)

    # hidden^T chunks: hT[b, ch, p, s] = hs[b, s, ch*128+p]
    hT = np.ascontiguousarray(
        hs.transpose(0, 2, 1).reshape(B, HCH, 128, S)).astype(BF16_NP)

    cos, sin, nsin = fc[0], fc[1], fc[2]      # [S, D]
    cosT = np.ascontiguousarray(cos.T).astype(BF16_NP)    # [128, S]
    csinT = np.concatenate([nsin.T[0:64], sin.T[64:128]], axis=0)
    csinT = np.ascontiguousarray(csinT).astype(BF16_NP)
    qw_col = np.ascontiguousarray(qnw.reshape(128, 1))
    kw_col = np.ascontiguousarray(knw.reshape(128, 1))

    # Wo^T chunks: wo[ic, p, hid] = Wo[hid, ic*128+p]
    woT = np.ascontiguousarray(Wo.T.reshape(ICH, 128, HID)).astype(BF16_NP)

    def prep_w(Wm, nblocks):
        # [nblocks, p, ch*128] with w[ob, p, ch*128+j] = Wm[ob*128+j, ch*128+p]
        a = Wm.reshape(nblocks, 128, HCH, 128).transpose(0, 3, 2, 1)
        return np.ascontiguousarray(a.reshape(nblocks, 128, HCH * 128)) \
            .astype(BF16_NP)

    in_maps = []
    for c in range(cores):
        Wq_c = Wq[c * HQ * D:(c + 1) * HQ * D]
        Wk_c = Wk[c * D:(c + 1) * D]
        Wv_c = Wv[c * D:(c + 1) * D]
        in_maps.append({
            "hT": hT,
            "wq": prep_w(Wq_c, HQ),
            "wk": prep_w(Wk_c, 1)[0],
            "wv": prep_w(Wv_c, 1)[0],
            "wo": woT,
            "cosT": cosT,
            "csinT": csinT,
            "qw": qw_col,
            "kw": kw_col,
        })
    return in_maps


def gather_output(results, B=2, S=1024, HID=4096, **_):
    cores = N_CORES
    TCB = (B * S) // cores // B
    out = np.empty((B, S, HID), dtype=np.float32)
    for c in range(cores):
        o = results[c]["out"]
        for b in range(B):
            out[b, c * TCB:(c + 1) * TCB] = o[b * TCB:(b + 1) * TCB]
    return out


_NC_CACHE = {}


def kernel(**inputs) -> np.ndarray:
    cfg = FULL_CFG
    key = tuple(sorted(cfg.items()))
    if key not in _NC_CACHE:
        _NC_CACHE[key] = build_program(**cfg)
    nc = _NC_CACHE[key]
    in_maps = host_prep(inputs, **cfg)
    res = run_bass_kernel_spmd(nc, in_maps, core_ids=list(range(N_CORES)))
    return gather_output(res.results, **cfg)



# revision 3
# speedup vs baseline: 1.1613x; 1.1613x over previous
"""Distributed Trainium2 (Bass/Tile) kernel for a Qwen3-style attention layer.

Full layer: QKV proj -> per-head RMSNorm (q,k) -> RoPE -> GQA SDPA -> o_proj.

Sharding over 8 NeuronCores:
  - tensor-parallel across heads for QKV+attention: core c owns q-heads
    [4c, 4c+4) and kv-head c; hidden_states replicated.
  - one AllGather per batch collects every core's attention context
    ([512, 1024] in -> [4096, 1024] out, rank-major rows = global ctx dims),
    then o_proj is OUTPUT-COLUMN sharded: core c computes all 2048 tokens
    x its 512 output columns with a resident 4.2MB Wo slice.  The host
    concatenates along the hidden axis.  No all-reduce needed.

Compute layout: everything lives transposed ([dim, token]) so the PE array
contracts over the partition axis with N=512 moving tiles in bf16.

Schedule: proj0 -> attn0 (ships ctx per head on the ACT DMA queue; one
AllGather fires at phase end and lands during proj1) -> proj1 -> attn1
(same; its AllGather latency is covered by oproj(0)) -> oproj(0) ->
oproj(1).  Wo-slice tiles + the first batch-0 ctx chunks prefetch on the
sync DMA queue during attn1, right after the proj pools free their SBUF.
"""

import numpy as np
import ml_dtypes

import concourse.bass as bass
import concourse.mybir as mybir
from concourse import bacc
from concourse.tile import TileContext
from concourse.bass_utils import run_bass_kernel_spmd
from concourse.masks import make_identity

F32 = mybir.dt.float32
BF16 = mybir.dt.bfloat16
BF16_NP = ml_dtypes.bfloat16

N_CORES = 8

FULL_CFG = dict(B=2, S=1024, HID=4096, H=32, KV=8, D=128, eps=1e-6)


def build_program(B=2, S=1024, HID=4096, H=32, KV=8, D=128, eps=1e-6):
    cores = N_CORES
    assert D == 128 and H % cores == 0 and KV == cores and B == 2
    HQ = H // cores            # q heads per core
    T = B * S                  # total tokens
    HCH = HID // 128           # hidden-dim chunks of 128
    TT = min(512, S)           # projection token tile (within batch)
    TPB = S // TT              # projection tiles per batch
    KB = S // 128              # key blocks per batch
    QT = min(512, S)           # attention q tile
    QTB = S // QT              # q tiles per batch
    ICH = (H * D) // 128       # o_proj contraction chunks (32)
    OCOL = HID // cores        # o_proj out cols per core (512)
    OB = OCOL // 128           # out blocks per core (4)
    scale = float(D) ** -0.5
    MULT = mybir.AluOpType.mult
    SW = QTB * QT              # full q row per batch (== S)
    RG = [list(range(cores))]

    nc = bacc.Bacc("TRN2", target_bir_lowering=False, debug=False,
                   num_devices=cores)

    hT = nc.dram_tensor("hT", [B, HCH, 128, S], BF16, kind="ExternalInput")
    wq = nc.dram_tensor("wq", [HQ, 128, HCH * 128], BF16, kind="ExternalInput")
    wk = nc.dram_tensor("wk", [128, HCH * 128], BF16, kind="ExternalInput")
    wv = nc.dram_tensor("wv", [128, HCH * 128], BF16, kind="ExternalInput")
    wo = nc.dram_tensor("wo", [ICH, 128, OCOL], BF16, kind="ExternalInput")
    cosT = nc.dram_tensor("cosT", [128, S], BF16, kind="ExternalInput")
    csinT = nc.dram_tensor("csinT", [128, S], BF16, kind="ExternalInput")
    qw = nc.dram_tensor("qw", [128, 1], F32, kind="ExternalInput")
    kw = nc.dram_tensor("kw", [128, 1], F32, kind="ExternalInput")
    out = nc.dram_tensor("out", [OB, 128, T], F32, kind="ExternalOutput")

    with TileContext(nc) as tc:
        with (
            tc.tile_pool(name="const", bufs=1) as cp,
            tc.tile_pool(name="dram", bufs=1, space="DRAM") as dramp,
            tc.tile_pool(name="qkv", bufs=1) as p_qkv,
            tc.tile_pool(name="work", bufs=2) as p_work,
            tc.tile_pool(name="pt", bufs=2) as p_pt,
            tc.tile_pool(name="psum", bufs=1, space="PSUM") as ps_all,
        ):
            ones_s = cp.tile([128, 128], BF16)
            nc.vector.memset(ones_s[:, :], 1.0)
            ident = cp.tile([128, 128], BF16)
            make_identity(nc, ident[:, :])
            eps_s = cp.tile([128, 1], F32)
            nc.vector.memset(eps_s[:, :], eps)
            cos_s = cp.tile([128, S], BF16)
            nc.sync.dma_start(out=cos_s[:, :], in_=cosT[:, :])
            csin_s = cp.tile([128, S], BF16)
            nc.sync.dma_start(out=csin_s[:, :], in_=csinT[:, :])
            qw_s = cp.tile([128, 1], F32)
            nc.sync.dma_start(out=qw_s[:, :], in_=qw[:, :])
            kw_s = cp.tile([128, 1], F32)
            nc.sync.dma_start(out=kw_s[:, :], in_=kw[:, :])

            ag_in = [dramp.tile([HQ * 128, S], BF16, tag=f"agi{b}",
                                name=f"agi{b}") for b in range(B)]
            ag_out = [dramp.tile([cores * HQ * 128, S], BF16,
                                 addr_space="Shared", tag=f"ago{b}",
                                 name=f"ago{b}") for b in range(B)]

            qT_s = p_qkv.tile([128, HQ * T], BF16, tag="qT")
            kT_s = p_qkv.tile([128, T], BF16, tag="kT")
            vnat_s = p_qkv.tile([128, T], BF16, tag="vnat")
            ctxT_s = p_qkv.tile([128, HQ * T], BF16, tag="ctxT")

            def proj(b, p_hid, p_w):
                """QKV projection + norm + rope for batch b."""
                w0_t = p_w.tile([128, HCH * 128], BF16, tag="w", name="w0")
                nc.sync.dma_start(out=w0_t[:, :], in_=wq[0])
                hch = []
                for ch in range(HCH):
                    t_ = p_hid.tile([128, S], BF16, tag="hid", name="hid")
                    nc.sync.dma_start(out=t_[:, :], in_=hT[b, ch, :, :])
                    hch.append(t_)
                for ob in range(HQ + 2):
                    if ob == 0:
                        w_t = w0_t
                    else:
                        w_t = p_w.tile([128, HCH * 128], BF16, tag="w",
                                       name="w")
                        srcw = (wq[ob] if ob < HQ else
                                (wk[:, :] if ob == HQ else wv[:, :]))
                        nc.sync.dma_start(out=w_t[:, :], in_=srcw)
                    for tt in range(TPB):
                        ps = ps_all.tile([128, TT], F32, tag="mm", name="ps", bufs=2, padded_shape=[128, SW])
                        for ch in range(HCH):
                            nc.tensor.matmul(
                                ps[:, :],
                                lhsT=w_t[:, ch * 128:(ch + 1) * 128],
                                rhs=hch[ch][:, tt * TT:(tt + 1) * TT],
                                start=(ch == 0), stop=(ch == HCH - 1))
                        tg = b * S + tt * TT
                        pos = tt * TT
                        if ob <= HQ:
                            is_q = ob < HQ
                            dst = (qT_s[:, ob * T + tg: ob * T + tg + TT]
                                   if is_q else kT_s[:, tg: tg + TT])
                            wcol = qw_s if is_q else kw_s
                            sq = p_work.tile([128, TT], BF16, tag="sq")
                            nc.scalar.square(sq[:, :], ps[:, :])
                            ssq = ps_all.tile([128, TT], F32, tag="aux", name="ssq", bufs=2)
                            nc.tensor.matmul(ssq[:, :], lhsT=ones_s[:, :],
                                             rhs=sq[:, :], start=True,
                                             stop=True)
                            std = p_work.tile([128, TT], F32, tag="std")
                            nc.scalar.activation(
                                std[:, :], ssq[:, :],
                                mybir.ActivationFunctionType.Sqrt,
                                bias=eps_s[:, :], scale=1.0 / D)
                            rs = p_work.tile([128, TT], F32, tag="rs")
                            nc.vector.reciprocal_approx_fast(rs[:, :],
                                                             std[:, :])
                            qn = p_work.tile([128, TT], F32, tag="qn")
                            nc.vector.scalar_tensor_tensor(
                                qn[:, :], in0=ps[:, :], scalar=wcol[:, :],
                                in1=rs[:, :], op0=MULT, op1=MULT)
                            qsw = p_work.tile([128, TT], F32, tag="qsw")
                            nc.sync.dma_start(out=qsw[0:64, :],
                                              in_=qn[64:128, :])
                            nc.sync.dma_start(out=qsw[64:128, :],
                                              in_=qn[0:64, :])
                            t1 = p_work.tile([128, TT], F32, tag="t1")
                            nc.vector.tensor_mul(t1[:, :], qn[:, :],
                                                 cos_s[:, pos: pos + TT])
                            t2 = p_work.tile([128, TT], BF16, tag="t2")
                            nc.vector.tensor_mul(t2[:, :], qsw[:, :],
                                                 csin_s[:, pos: pos + TT])
                            nc.vector.tensor_add(dst, t1[:, :], t2[:, :])
                        else:
                            vt = p_work.tile([128, TT], BF16, tag="vt")
                            nc.scalar.copy(vt[:, :], ps[:, :])
                            for tb in range(TT // 128):
                                vtr = ps_all.tile([128, 128], BF16, tag="aux", name="vtr", bufs=2)
                                nc.tensor.transpose(
                                    vtr[:, :], vt[:, tb * 128:(tb + 1) * 128],
                                    ident[:, :])
                                tbg = tg // 128 + tb
                                nc.scalar.copy(
                                    vnat_s[:, tbg * 128:(tbg + 1) * 128],
                                    vtr[:, :])

            def attn(b):
                """Attention for batch b; ships ctx per head, one AG at end."""
                for h in range(HQ):
                    qoff = h * T + b * S
                    pt_t = p_pt.tile([128, KB * SW], BF16, tag="pT",
                                     name="pT")
                    for kb in range(KB):
                        sps = ps_all.tile([128, SW], F32, tag="mm", name="sps", bufs=2)
                        for qt in range(QTB):
                            nc.tensor.matmul(
                                sps[:, qt * QT:(qt + 1) * QT],
                                lhsT=kT_s[:, b * S + kb * 128:
                                          b * S + (kb + 1) * 128],
                                rhs=qT_s[:, qoff + qt * QT:
                                         qoff + (qt + 1) * QT],
                                start=True, stop=True)
                        nc.scalar.activation(
                            pt_t[:, kb * SW:(kb + 1) * SW], sps[:, :],
                            mybir.ActivationFunctionType.Exp, scale=scale)
                    ctxs = [ps_all.tile([128, QT], F32, tag="ctx", name="ctx",
                                     bufs=2) for _ in range(QTB)]
                    for kb in range(KB):
                        tbg = (b * S) // 128 + kb
                        for qt in range(QTB):
                            nc.tensor.matmul(
                                ctxs[qt][:, :],
                                lhsT=vnat_s[:, tbg * 128:(tbg + 1) * 128],
                                rhs=pt_t[:, kb * SW + qt * QT:
                                         kb * SW + (qt + 1) * QT],
                                start=(kb == 0), stop=(kb == KB - 1))
                    denp = p_work.tile([128, SW], BF16, tag="denp")
                    dent = p_work.tile([128, SW], BF16, tag="dent")
                    if KB == 2:
                        nc.vector.tensor_add(denp[:, :], pt_t[:, 0:SW],
                                             pt_t[:, SW:2 * SW])
                    else:
                        assert KB % 4 == 0
                        nc.vector.tensor_add(denp[:, :], pt_t[:, 0:SW],
                                             pt_t[:, SW:2 * SW])
                        nc.vector.tensor_add(dent[:, :],
                                             pt_t[:, 2 * SW:3 * SW],
                                             pt_t[:, 3 * SW:4 * SW])
                        nc.vector.tensor_add(denp[:, :], denp[:, :],
                                             dent[:, :])
                        for g in range(1, KB // 4):
                            nc.vector.tensor_add(
                                dent[:, :],
                                pt_t[:, 4 * g * SW:(4 * g + 1) * SW],
                                pt_t[:, (4 * g + 1) * SW:(4 * g + 2) * SW])
                            nc.vector.tensor_add(denp[:, :], denp[:, :],
                                                 dent[:, :])
                            nc.vector.tensor_add(
                                dent[:, :],
                                pt_t[:, (4 * g + 2) * SW:(4 * g + 3) * SW],
                                pt_t[:, (4 * g + 3) * SW:(4 * g + 4) * SW])
                            nc.vector.tensor_add(denp[:, :], denp[:, :],
                                                 dent[:, :])
                    for qt in range(QTB):
                        dps = ps_all.tile([128, QT], F32, tag="aux", name="dps", bufs=2)
                        nc.tensor.matmul(dps[:, :], lhsT=ones_s[:, :],
                                         rhs=denp[:, qt * QT:(qt + 1) * QT],
                                         start=True, stop=True)
                        rec = p_work.tile([128, QT], F32, tag="rec")
                        nc.vector.reciprocal_approx_fast(rec[:, :], dps[:, :])
                        nc.vector.tensor_mul(
                            ctxT_s[:, qoff + qt * QT: qoff + (qt + 1) * QT],
                            ctxs[qt][:, :], rec[:, :])
                    # ship this head's context (ACT-queue DMA so the sync
                    # queue stays free for Wo/ctx prefetch during attn1)
                    nc.scalar.dma_start(
                        out=ag_in[b][h * 128:(h + 1) * 128, :],
                        in_=ctxT_s[:, qoff: qoff + S])
                nc.gpsimd.collective_compute(
                    "AllGather", mybir.AluOpType.bypass,
                    replica_groups=RG,
                    ins=[ag_in[b].opt()],
                    outs=[ag_out[b].opt()])

            # ---- phase 1: proj0, attn0 (+AG0), proj1, attn1 (+AG1) -------
            with (
                tc.tile_pool(name="hid", bufs=HCH) as p_hid,
                tc.tile_pool(name="wts", bufs=2) as p_w,
            ):
                sc_ = nc.enter_named_scope("proj0", True)[0]
                proj(0, p_hid, p_w)
                nc.leave_named_scope("proj0", sc_, True)
                sc_ = nc.enter_named_scope("attn0", True)[0]
                attn(0)
                nc.leave_named_scope("attn0", sc_, True)
                sc_ = nc.enter_named_scope("proj1", True)[0]
                proj(1, p_hid, p_w)
                nc.leave_named_scope("proj1", sc_, True)

            # phase-2 SBUF lives where hid/wts were; prefetch DMAs emitted
            # before attn1 so they drain during it (WAR on proj1's last
            # reads resolves as soon as proj1's matmuls retire).
            with (
                tc.tile_pool(name="wo", bufs=ICH) as p_wo,
                tc.tile_pool(name="cx", bufs=12) as p_cx,
                tc.tile_pool(name="oo", bufs=4) as p_oo,
            ):
                wo_ts = []
                for g in range(ICH):
                    wo_t = p_wo.tile([128, OCOL], BF16, tag="wo",
                                     name="wo")
                    nc.sync.dma_start(out=wo_t[:, :], in_=wo[g])
                    wo_ts.append(wo_t)
                cx_ts = {}
                NPRE = 10

                def load_cx(b, g):
                    cx_t = p_cx.tile([128, S], BF16, tag="cx", name="cx")
                    nc.sync.dma_start(
                        out=cx_t[:, :],
                        in_=ag_out[b][g * 128:(g + 1) * 128, :])
                    cx_ts[(b, g)] = cx_t

                for g in range(NPRE):
                    load_cx(0, g)

                sc_ = nc.enter_named_scope("attn1", True)[0]
                attn(1)
                nc.leave_named_scope("attn1", sc_, True)

                sc_ = nc.enter_named_scope("oproj", True)[0]

                def oproj(b):
                    mmA = ps_all.tile([128, SW], F32, tag="mm",
                                      name="mmA", bufs=2)
                    mmB = ps_all.tile([128, SW], F32, tag="mm",
                                      name="mmB", bufs=2)
                    c0 = ps_all.tile([128, QT], F32, tag="ctx",
                                     name="c0", bufs=2)
                    c1 = ps_all.tile([128, QT], F32, tag="ctx",
                                     name="c1", bufs=2)
                    a0 = ps_all.tile([128, TT], F32, tag="aux",
                                     name="a0", bufs=2)
                    a1 = ps_all.tile([128, TT], F32, tag="aux",
                                     name="a1", bufs=2)
                    accs = {(0, 0): mmA[:, 0:QT], (0, 1): mmA[:, QT:2 * QT],
                            (1, 0): mmB[:, 0:QT], (1, 1): mmB[:, QT:2 * QT],
                            (2, 0): c0[:, :], (2, 1): c1[:, :],
                            (3, 0): a0[:, :], (3, 1): a1[:, :]}
                    for g in range(ICH):
                        if (b, g) not in cx_ts:
                            load_cx(b, g)
                        cx_t = cx_ts[(b, g)]
                        for ob in range(OB):
                            for t2 in range(2):
                                nc.tensor.matmul(
                                    accs[(ob, t2)],
                                    lhsT=wo_ts[g][:, ob * 128:
                                                  (ob + 1) * 128],
                                    rhs=cx_t[:, t2 * QT:(t2 + 1) * QT],
                                    start=(g == 0), stop=(g == ICH - 1))
                    # prefetch next batch's first ctx chunks before the
                    # out-writes occupy the sync queue
                    if b + 1 < B:
                        for g in range(8):
                            load_cx(b + 1, g)
                    for ob in range(OB):
                        ot = p_oo.tile([128, S], F32, tag="oo", name="ot")
                        nc.vector.tensor_copy(ot[:, 0:QT], accs[(ob, 0)])
                        nc.vector.tensor_copy(ot[:, QT:2 * QT],
                                              accs[(ob, 1)])
                        nc.sync.dma_start(
                            out=out[ob][:, b * S:(b + 1) * S],
                            in_=ot[:, :])

                for b in range(B):
                    oproj(b)
                nc.leave_named_scope("oproj", sc_, True)

    nc.compile()
    return nc


def host_prep(inputs, B=2, S=1024, HID=4096, H=32, KV=8, D=128, eps=1e-6):
    """Shard + lay out the full inputs into per-core in_maps."""
    cores = N_CORES
    HQ = H // cores
    HCH = HID // 128
    ICH = (H * D) // 128
    OCOL = HID // cores

    hs = np.ascontiguousarray(inputs["hidden_states"], dtype=np.float32)
    fc = np.asarray(inputs["freqs_cis"], dtype=np.float32)
    Wq = np.asarray(inputs["Wq"], dtype=np.float32)
    Wk = np.asarray(inputs["Wk"], dtype=np.float32)
    Wv = np.asarray(inputs["Wv"], dtype=np.float32)
    Wo = np.asarray(inputs["Wo"], dtype=np.float32)
    qnw = np.asarray(inputs["q_norm_w"], dtype=np.float32)
    knw = np.asarray(inputs["k_norm_w"], dtype=np.float32)

    # hidden^T chunks: hT[b, ch, p, s] = hs[b, s, ch*128+p]
    hT = np.ascontiguousarray(
        hs.transpose(0, 2, 1).reshape(B, HCH, 128, S)).astype(BF16_NP)

    cos, sin, nsin = fc[0], fc[1], fc[2]      # [S, D]
    cosT = np.ascontiguousarray(cos.T).astype(BF16_NP)    # [128, S]
    csinT = np.concatenate([nsin.T[0:64], sin.T[64:128]], axis=0)
    csinT = np.ascontiguousarray(csinT).astype(BF16_NP)
    qw_col = np.ascontiguousarray(qnw.reshape(128, 1))
    kw_col = np.ascontiguousarray(knw.reshape(128, 1))

    def prep_w(Wm, nblocks):
        # [nblocks, p, ch*128] with w[ob, p, ch*128+j] = Wm[ob*128+j, ch*128+p]
        a = Wm.reshape(nblocks, 128, HCH, 128).transpose(0, 3, 2, 1)
        return np.ascontiguousarray(a.reshape(nblocks, 128, HCH * 128)) \
            .astype(BF16_NP)

    in_maps = []
    for c in range(cores):
        Wq_c = Wq[c * HQ * D:(c + 1) * HQ * D]
        Wk_c = Wk[c * D:(c + 1) * D]
        Wv_c = Wv[c * D:(c + 1) * D]
        # Wo slice: core c owns out cols [c*OCOL, (c+1)*OCOL)
        # woc[ic, p, j] = Wo[c*OCOL + j, ic*128 + p]
        woc = np.ascontiguousarray(
            Wo[c * OCOL:(c + 1) * OCOL, :].reshape(OCOL, ICH, 128)
            .transpose(1, 2, 0)).astype(BF16_NP)
        in_maps.append({
            "hT": hT,
            "wq": prep_w(Wq_c, HQ),
            "wk": prep_w(Wk_c, 1)[0],
            "wv": prep_w(Wv_c, 1)[0],
            "wo": woc,
            "cosT": cosT,
            "csinT": csinT,
            "qw": qw_col,
            "kw": kw_col,
        })
    return in_maps


def gather_output(results, B=2, S=1024, HID=4096, **_):
    cores = N_CORES
    OCOL = HID // cores
    out = np.empty((B, S, HID), dtype=np.float32)
    for c in range(cores):
        o = results[c]["out"]                      # [OB, 128, B*S]
        o = o.reshape(OCOL, B, S).transpose(1, 2, 0)
        out[:, :, c * OCOL:(c + 1) * OCOL] = o
    return out


_NC_CACHE = {}


def kernel(**inputs) -> np.ndarray:
    cfg = FULL_CFG
    key = tuple(sorted(cfg.items()))
    if key not in _NC_CACHE:
        _NC_CACHE[key] = build_program(**cfg)
    nc = _NC_CACHE[key]
    in_maps = host_prep(inputs, **cfg)
    res = run_bass_kernel_spmd(nc, in_maps, core_ids=list(range(N_CORES)))
    return gather_output(res.results, **cfg)
